# revision 1
# baseline (speedup 1.0000x reference)
"""CrossSymmetricModal trn2 kernel v3: v2 structure, baseline-proven ops only.

Data-parallel over batch (2 samples/core on 8 cores); BatchNorm batch stats
cross-core via a small AllReduce per branch.

 - all q/k/v convs for all 4 (branch, sample) iterations hoisted up front so
   the attention matmul stream never waits on conv deps.
 - softmax denominator via an all-ones [128,128] stationary matmul (den
   arrives broadcast on every partition); vb/den-bcast matmuls eliminated
   (v-bias folded into v tiles via an im2col ones-row / a fused DVE add).
 - input DMAs ordered so the first conv's weights+inputs land first and the
   f32 residual image (only needed at the BN tail) lands last.
 - branch-a BN finalize + output DMAs are emitted before the branch-b stats
   load so they overlap the branch-b AllReduce.
"""
import os
import sys

sys.path.insert(0, '/opt/trn_rl_repo')

import ml_dtypes
import numpy as np

from concourse import bacc, mybir, tile
from concourse.bass_utils import run_bass_kernel_spmd

S = 2
NCORES = 8
C = 256
CT = 2
L = 1024
LS = 2
EPS = 1e-5
SCALE = 1.0 / 16.0
NSTAT = 16 * L

F32 = mybir.dt.float32
F32R = mybir.dt.float32r
BF16 = mybir.dt.bfloat16
NPBF = ml_dtypes.bfloat16
AF = mybir.ActivationFunctionType
OP = mybir.AluOpType
AXX = mybir.AxisListType.X

RECIP_FAST = os.environ.get("KERNEL_RECIP", "fast") == "fast"

_NC_CACHE = []


def _build_nc():
    nc = bacc.Bacc(num_devices=NCORES)

    imm_p = nc.declare_dram_parameter("imm", [S, CT, 128, L + 2], BF16, isOutput=False)
    img_p = nc.declare_dram_parameter("img", [S, CT, 128, L], F32, isOutput=False)
    cli_p = nc.declare_dram_parameter("cli", [S, 4, L], BF16, isOutput=False)
    w_cc_p = {}
    for name in ("wq_a", "wo_a", "wk_b", "wo_b"):
        w_cc_p[name] = nc.declare_dram_parameter(name, [CT, 128, 768], BF16, isOutput=False)
    wv_b_p = nc.declare_dram_parameter("wv_b", [CT, 3, 128, C], BF16, isOutput=False)
    w_sm_p = {}
    for name in ("wk_a", "wv_a", "wq_b"):
        w_sm_p[name] = nc.declare_dram_parameter(name, [4, C], BF16, isOutput=False)
    bias_p = {}
    for name in ("qb_a", "kb_b", "ob_a", "ob_b"):
        bias_p[name] = nc.declare_dram_parameter(name, [128, CT], F32, isOutput=False)
    vbb_p = nc.declare_dram_parameter("vbb", [128, C], F32, isOutput=False)
    gb_p = {}
    for name in ("ga_a", "be_a", "ga_b", "be_b"):
        gb_p[name] = nc.declare_dram_parameter(name, [128, CT], F32, isOutput=False)
    out_p = nc.declare_dram_parameter("out", [S, 2, CT, 128, L], F32, isOutput=True)

    from contextlib import ExitStack
    with tile.TileContext(nc) as tc, ExitStack() as es:
        ec = es.enter_context
        wgt = ec(tc.tile_pool(name="wgt", bufs=1))
        io = ec(tc.tile_pool(name="io", bufs=1))
        qk = ec(tc.tile_pool(name="qk", bufs=1))
        vtp = ec(tc.tile_pool(name="vtp", bufs=1))
        ex = ec(tc.tile_pool(name="ex", bufs=2))
        cx = ec(tc.tile_pool(name="cx", bufs=2))
        op_pool = ec(tc.tile_pool(name="op", bufs=1))
        sm = ec(tc.tile_pool(name="sm", bufs=2))
        sqp = ec(tc.tile_pool(name="sqp", bufs=2))
        st = ec(tc.tile_pool(name="st", bufs=1))
        bn = ec(tc.tile_pool(name="bn", bufs=3))
        dram = ec(tc.tile_pool(name="dram", bufs=1, space="DRAM"))
        ps_conv = ec(tc.tile_pool(name="psc", bufs=2, space="PSUM"))
        ps_sc = ec(tc.tile_pool(name="pss", bufs=3, space="PSUM"))
        ps_ctx = ec(tc.tile_pool(name="psx", bufs=1, space="PSUM"))
        ps_den = ec(tc.tile_pool(name="psd", bufs=1, space="PSUM"))
        if True:
            # ---- DMAs, ordered: first conv's deps first, residual img last ----
            w_cc = {}
            w_cc["wq_a"] = []
            for kt in range(CT):
                t_ = wgt.tile([128, 768], BF16, tag=f"wq_a_{kt}")
                nc.sync.dma_start(out=t_, in_=w_cc_p["wq_a"][kt])
                w_cc["wq_a"].append(t_)
            imm = []
            cli = []
            for s in range(S):
                imm.append([])
                for kt in range(CT):
                    t_ = io.tile([128, L + 2], BF16, tag=f"imm_{s}_{kt}")
                    nc.sync.dma_start(out=t_, in_=imm_p[s, kt])
                    imm[s].append(t_)
                t_ = io.tile([4, L], BF16, tag=f"cli_{s}")
                nc.sync.dma_start(out=t_, in_=cli_p[s])
                cli.append(t_)
            vbb_bc = wgt.tile([128, C], F32, tag="vbb_bc")
            nc.sync.dma_start(out=vbb_bc, in_=vbb_p[:, :])
            bias = {}
            for name in ("qb_a", "kb_b", "ob_a", "ob_b"):
                t_ = wgt.tile([128, CT], F32, tag=name)
                nc.sync.dma_start(out=t_, in_=bias_p[name][:, :])
                bias[name] = t_
            gb = {}
            for name in ("ga_a", "be_a", "ga_b", "be_b"):
                t_ = wgt.tile([128, CT], F32, tag=name)
                nc.sync.dma_start(out=t_, in_=gb_p[name][:, :])
                gb[name] = t_
            w_sm = {}
            for name in ("wk_a", "wv_a", "wq_b"):
                t_ = wgt.tile([4, C], BF16, tag=name)
                nc.sync.dma_start(out=t_, in_=w_sm_p[name][:, :])
                w_sm[name] = t_
            for name in ("wk_b", "wo_a", "wo_b"):
                w_cc[name] = []
                for kt in range(CT):
                    t_ = wgt.tile([128, 768], BF16, tag=f"{name}_{kt}")
                    nc.sync.dma_start(out=t_, in_=w_cc_p[name][kt])
                    w_cc[name].append(t_)
            wv_b = []
            for kt in range(CT):
                row = []
                for t in range(3):
                    t_ = wgt.tile([128, C], BF16, tag=f"wv_b_{kt}_{t}")
                    nc.sync.dma_start(out=t_, in_=wv_b_p[kt, t])
                    row.append(t_)
                wv_b.append(row)
            img = []
            for s in range(S):
                img.append([])
                for kt in range(CT):
                    t_ = io.tile([128, L], F32, tag=f"img_{s}_{kt}")
                    nc.sync.dma_start(out=t_, in_=img_p[s, kt])
                    img[s].append(t_)

            # ---- constants ----
            ones_full_f = wgt.tile([128, 128], F32, tag="ones_full_f")
            nc.vector.memset(ones_full_f, 1.0)
            ones_full = wgt.tile([128, 128], BF16, tag="ones_full")
            nc.vector.tensor_copy(out=ones_full, in_=ones_full_f)
            eps_sb = wgt.tile([128, 1], F32, tag="eps_sb")
            nc.vector.memset(eps_sb, EPS)
            zero_col = wgt.tile([128, 1], BF16, tag="zero_col")
            nc.vector.memset(zero_col, 0.0)

            # ---- conv phase ----
            def conv_cc(dst, w_kt, bias_ap, src):
                for ct in range(CT):
                    for ls in range(LS):
                        p = ps_conv.tile([128, 512], F32, tag="conv", name="convp")
                        n = 0
                        for kt in range(CT):
                            for t in range(3):
                                nc.tensor.matmul(
                                    p,
                                    lhsT=w_kt[kt][:, (t * 2 + ct) * 128:(t * 2 + ct + 1) * 128],
                                    rhs=src[kt][:, ls * 512 + t: ls * 512 + t + 512],
                                    start=(n == 0), stop=(n == 5))
                                n += 1
                        nc.scalar.activation(
                            out=dst[ct][:, ls * 512:(ls + 1) * 512], in_=p,
                            func=AF.Identity, bias=bias_ap[:, ct:ct + 1], scale=1.0)

            def conv_1c(dst, w_lhsT, cli_t):
                for ct in range(CT):
                    for ls in range(LS):
                        p = ps_conv.tile([128, 512], F32, tag="conv", name="convp")
                        nc.tensor.matmul(
                            p, lhsT=w_lhsT[:, ct * 128:(ct + 1) * 128],
                            rhs=cli_t[:, ls * 512:(ls + 1) * 512],
                            start=True, stop=True)
                        nc.scalar.activation(
                            out=dst[ct][:, ls * 512:(ls + 1) * 512], in_=p,
                            func=AF.Identity)

            q_sb = {}
            k_sb = {}
            vt = {}
            for br in range(2):
                for s in range(S):
                    q_sb[(br, s)] = [qk.tile([128, L], BF16, tag=f"q{br}_{s}_{kt}", name=f"q{br}_{s}_{kt}") for kt in range(CT)]
                    k_sb[(br, s)] = [qk.tile([128, L], BF16, tag=f"k{br}_{s}_{kt}", name=f"k{br}_{s}_{kt}") for kt in range(CT)]
                    vt[(br, s)] = [vtp.tile([128, C], BF16, tag=f"vt{br}_{s}_{mt}", name=f"vt{br}_{s}_{mt}") for mt in range(8)]

            # branch-a convs for BOTH samples first: they only need wq_a +
            # imm + cli (the first DMAs), giving the DMA stream ~19us of PE
            # work to land the branch-b weights (wk_b/wv_b)
            for s in range(S):
                conv_cc(q_sb[(0, s)], w_cc["wq_a"], bias["qb_a"], imm[s])
                conv_1c(k_sb[(0, s)], w_sm["wk_a"], cli[s])
                for mt in range(8):
                    p = ps_conv.tile([128, C], F32, tag="conv", name="convp")
                    nc.tensor.matmul(
                        p, lhsT=cli[s][:, mt * 128:(mt + 1) * 128],
                        rhs=w_sm["wv_a"], start=True, stop=True)
                    nc.vector.tensor_copy(out=vt[(0, s)][mt], in_=p)
            for s in range(S):
                conv_1c(q_sb[(1, s)], w_sm["wq_b"], cli[s])
                conv_cc(k_sb[(1, s)], w_cc["wk_b"], bias["kb_b"], imm[s])
                for mt in range(8):
                    p = ps_conv.tile([128, C], F32, tag="conv", name="convp")
                    n = 0
                    for kt in range(CT):
                        for t in range(3):
                            nc.tensor.matmul(
                                p,
                                lhsT=imm[s][kt][:, mt * 128 + t: mt * 128 + t + 128],
                                rhs=wv_b[kt][t],
                                start=(n == 0), stop=(n == 5))
                            n += 1
                    nc.vector.tensor_add(out=vt[(1, s)][mt], in0=p, in1=vbb_bc)

            # ---- attention + out conv ----
            o_tiles = {}
            slots = {}
            statg = {}
            for br in range(2):
                slots[br] = st.tile([128, 4 * S * LS], F32, tag=f"slots{br}", name=f"slots{br}")

            def attention(br, s):
                qs, ks, vs = q_sb[(br, s)], k_sb[(br, s)], vt[(br, s)]
                ctx = [cx.tile([128, L + 2], BF16, tag=f"ctx{ct}", name=f"ctx{ct}") for ct in range(CT)]
                for ct in range(CT):
                    nc.vector.tensor_copy(out=ctx[ct][:, 0:1], in_=zero_col)
                    nc.vector.tensor_copy(out=ctx[ct][:, L + 1:L + 2], in_=zero_col)
                for ls in range(LS):
                    ets = {}

                    def _sc_exp(mt):
                        sc = ps_sc.tile([128, 512], F32, tag="sc", name="sc")
                        for kt in range(CT):
                            nc.tensor.matmul(
                                sc, lhsT=ks[kt][:, mt * 128:(mt + 1) * 128],
                                rhs=qs[kt][:, ls * 512:(ls + 1) * 512],
                                start=(kt == 0), stop=(kt == CT - 1))
                        et = ex.tile([128, 512], BF16, tag=f"et{mt}", name="et")
                        nc.scalar.activation(out=et, in_=sc, func=AF.Exp, scale=SCALE)
                        ets[mt] = et

                    ctx_ps = [ps_ctx.tile([128, 512], F32, tag=f"ctxp{ct}", name=f"ctxp{ct}") for ct in range(CT)]
                    den_ps = ps_den.tile([128, 512], F32, tag="den", name="den_ps")
                    _sc_exp(0)
                    _sc_exp(1)
                    _sc_exp(2)
                    for mt in range(8):
                        if mt + 3 < 8:
                            _sc_exp(mt + 3)
                        et = ets.pop(mt)
                        for ct in range(CT):
                            nc.tensor.matmul(
                                ctx_ps[ct], lhsT=vs[mt][:, ct * 128:(ct + 1) * 128],
                                rhs=et, start=(mt == 0), stop=(mt == 7))
                        nc.tensor.matmul(
                            den_ps, lhsT=ones_full, rhs=et,
                            start=(mt == 0), stop=(mt == 7))
                    recip = sm.tile([128, 512], F32, tag="recip", name="recip")
                    if RECIP_FAST:
                        nc.vector.reciprocal_approx_fast(out=recip, in_=den_ps)
                    else:
                        nc.vector.reciprocal(out=recip, in_=den_ps)
                    for ct in range(CT):
                        nc.vector.tensor_mul(
                            out=ctx[ct][:, 1 + ls * 512: 1 + (ls + 1) * 512],
                            in0=ctx_ps[ct], in1=recip)
                return ctx

            def out_conv(br, s, ctx):
                abr = "a" if br == 0 else "b"
                o_sb = [op_pool.tile([128, L], F32, tag=f"o_{br}_{s}_{ct}", name=f"o_{br}_{s}_{ct}") for ct in range(CT)]
                for ct in range(CT):
                    o_tiles[(br, s, ct)] = o_sb[ct]
                for ct in range(CT):
                    for ls in range(LS):
                        p = ps_conv.tile([128, 512], F32, tag="conv", name="convp")
                        n = 0
                        for kt in range(CT):
                            for t in range(3):
                                nc.tensor.matmul(
                                    p,
                                    lhsT=w_cc[f"wo_{abr}"][kt][:, (t * 2 + ct) * 128:(t * 2 + ct + 1) * 128],
                                    rhs=ctx[kt][:, ls * 512 + t: ls * 512 + t + 512],
                                    start=(n == 0), stop=(n == 5))
                                n += 1
                        osl = o_sb[ct][:, ls * 512:(ls + 1) * 512]
                        i = ct * S * LS + s * LS + ls
                        nc.scalar.activation(
                            out=osl, in_=p, func=AF.Identity,
                            bias=bias[f"ob_{abr}"][:, ct:ct + 1], scale=1.0,
                            accum_out=slots[br][:, i:i + 1])
                        sq = sqp.tile([128, 512], F32, tag="sq", name="sq")
                        nc.vector.tensor_mul(out=sq, in0=osl, in1=osl)
                        j = (2 + ct) * S * LS + s * LS + ls
                        nc.vector.reduce_sum(out=slots[br][:, j:j + 1], in_=sq, axis=AXX)

            cc_outs = {}
            for br in range(2):
                for s in range(S):
                    ctx = attention(br, s)
                    out_conv(br, s, ctx)
                statp = st.tile([128, 4], F32, tag=f"statp{br}", name=f"statp{br}")
                nc.vector.reduce_sum(
                    out=statp,
                    in_=slots[br].rearrange("p (g i) -> p g i", i=S * LS), axis=AXX)
                cc_in = dram.tile([128, 4], F32, tag=f"ccin{br}", name=f"ccin{br}")
                cc_out = dram.tile([128, 4], F32, tag=f"ccout{br}", name=f"ccout{br}")
                nc.sync.dma_start(out=cc_in, in_=statp)
                if os.environ.get("KERNEL_NO_CC"):
                    nc.sync.dma_start(out=cc_out, in_=cc_in)
                else:
                    nc.gpsimd.collective_compute(
                        "AllReduce", OP.add,
                        replica_groups=[list(range(NCORES))],
                        ins=[cc_in.opt()], outs=[cc_out.opt()])
                cc_outs[br] = cc_out
                if br == 0:
                    sg = st.tile([128, 4], F32, tag="statg0", name="statg0")
                    nc.sync.dma_start(out=sg, in_=cc_out)
                    statg[0] = sg

            # ---- BN finalize + residual + output ----
            def bn_finalize(br):
                abr = "a" if br == 0 else "b"
                sg = statg[br]
                mean = st.tile([128, CT], F32, tag=f"mean{br}", name=f"mean{br}")
                nc.vector.tensor_scalar_mul(mean, sg[:, 0:2], 1.0 / NSTAT)
                esq = st.tile([128, CT], F32, tag=f"esq{br}", name=f"esq{br}")
                nc.vector.tensor_scalar_mul(esq, sg[:, 2:4], 1.0 / NSTAT)
                m2 = st.tile([128, CT], F32, tag=f"m2{br}", name=f"m2{br}")
                nc.vector.tensor_mul(out=m2, in0=mean, in1=mean)
                var = st.tile([128, CT], F32, tag=f"var{br}", name=f"var{br}")
                nc.vector.tensor_sub(out=var, in0=esq, in1=m2)
                sd = st.tile([128, CT], F32, tag=f"sd{br}", name=f"sd{br}")
                nc.scalar.activation(out=sd, in_=var, func=AF.Sqrt, bias=eps_sb[:, 0:1], scale=1.0)
                rstd = st.tile([128, CT], F32, tag=f"rstd{br}", name=f"rstd{br}")
                nc.vector.reciprocal(out=rstd, in_=sd)
                A_ = st.tile([128, CT], F32, tag=f"A{br}", name=f"A{br}")
                nc.vector.tensor_mul(out=A_, in0=rstd, in1=gb[f"ga_{abr}"])
                mA = st.tile([128, CT], F32, tag=f"mA{br}", name=f"mA{br}")
                nc.vector.tensor_mul(out=mA, in0=mean, in1=A_)
                Bc = st.tile([128, CT], F32, tag=f"Bc{br}", name=f"Bc{br}")
                nc.vector.tensor_sub(out=Bc, in0=gb[f"be_{abr}"], in1=mA)
                for s in range(S):
                    for ct in range(CT):
                        for ls in range(LS):
                            tmp = bn.tile([128, 512], F32, tag="bnt", name="bnt")
                            nc.scalar.activation(
                                out=tmp,
                                in_=o_tiles[(br, s, ct)][:, ls * 512:(ls + 1) * 512],
                                func=AF.Identity,
                                scale=A_[:, ct:ct + 1], bias=Bc[:, ct:ct + 1])
                            res = bn.tile([128, 512], F32, tag="bnr", name="bnr")
                            nc.vector.tensor_add(
                                out=res, in0=tmp,
                                in1=img[s][ct][:, ls * 512:(ls + 1) * 512])
                            nc.sync.dma_start(
                                out=out_p[s, br, ct, :, ls * 512:(ls + 1) * 512], in_=res)

            bn_finalize(0)
            sg1 = st.tile([128, 4], F32, tag="statg1", name="statg1")
            nc.sync.dma_start(out=sg1, in_=cc_outs[1])
            statg[1] = sg1
            bn_finalize(1)

    nc.compile()
    return nc


def _get_nc():
    if not _NC_CACHE:
        _NC_CACHE.append(_build_nc())
    return _NC_CACHE[0]


def _prep_shared(inp):
    f = NPBF
    m = {}

    def cc_layout(w):
        return np.ascontiguousarray(
            np.asarray(w).reshape(2, 128, 2, 128, 3).transpose(2, 3, 4, 0, 1).reshape(2, 128, 768)).astype(f)

    m["wq_a"] = cc_layout(inp["a_qw"])
    m["wo_a"] = cc_layout(inp["a_ow"])
    m["wk_b"] = cc_layout(inp["b_kw"])
    m["wo_b"] = cc_layout(inp["b_ow"])
    m["wv_b"] = np.ascontiguousarray(
        np.asarray(inp["b_vw"]).reshape(C, 2, 128, 3).transpose(1, 3, 2, 0)).astype(f)

    def sm_layout(w, b):
        w3 = np.asarray(w)[:, 0, :].T
        return np.ascontiguousarray(
            np.concatenate([w3, np.asarray(b)[None, :]], axis=0)).astype(f)

    m["wk_a"] = sm_layout(inp["a_kw"], inp["a_kb"])
    m["wv_a"] = sm_layout(inp["a_vw"], inp["a_vb"])
    m["wq_b"] = sm_layout(inp["b_qw"], inp["b_qb"])
    for dst, src in (("qb_a", "a_qb"), ("kb_b", "b_kb"), ("ob_a", "a_ob"),
                     ("ob_b", "b_ob"),
                     ("ga_a", "a_g"), ("be_a", "a_beta"),
                     ("ga_b", "b_g"), ("be_b", "b_beta")):
        m[dst] = np.ascontiguousarray(np.asarray(inp[src]).reshape(2, 128).T).astype(np.float32)
    m["vbb"] = np.ascontiguousarray(
        np.repeat(np.asarray(inp["b_vb"])[None, :], 128, axis=0)).astype(np.float32)
    return m


def _core_maps(image, clinical, shared, ncores=NCORES):
    in_maps = []
    for core in range(ncores):
        m = dict(shared)
        sl = slice(core * S, (core + 1) * S)
        a = image[sl].reshape(S, CT, 128, L)
        pad = np.zeros((S, CT, 128, L + 2), np.float32)
        pad[..., 1:L + 1] = a
        m["imm"] = pad.astype(NPBF)
        m["img"] = np.ascontiguousarray(a).astype(np.float32)
        c = clinical[sl][:, 0, :]
        im2 = np.zeros((S, 4, L), np.float32)
        im2[:, 0, 1:] = c[:, :L - 1]
        im2[:, 1, :] = c
        im2[:, 2, :L - 1] = c[:, 1:]
        im2[:, 3, :] = 1.0
        m["cli"] = im2.astype(NPBF)
        in_maps.append(m)
    return in_maps


def kernel(**inputs):
    inp = {k: np.asarray(v) for k, v in inputs.items()}
    nc = _get_nc()
    shared = _prep_shared(inp)
    image = inp["image"].astype(np.float32)
    clinical = inp["clinical"].astype(np.float32)
    in_maps = _core_maps(image, clinical, shared)
    res = run_bass_kernel_spmd(nc, in_maps, core_ids=list(range(NCORES)))
    outs = np.concatenate([res.results[i]["out"] for i in range(NCORES)], axis=0)
    return np.ascontiguousarray(outs.reshape(16, 512, L))



# revision 3
# speedup vs baseline: 1.2493x; 1.2493x over previous
"""CrossSymmetricModal trn2 kernel v4: rank-4 factorization of clinical ops.

Key idea: clinical is [B, 1, L], so every conv taking it as input has rank-4
structure over the im2col features phi = (cli[m-1], cli[m], cli[m+1], 1).

- branch a (q=image, k/v=clinical): k = Wk^T phi, v = Wv^T phi.
  scores = g.phi with g = Wk.q computed DIRECTLY as a 4-row conv of image
  (weights folded on host). ctx collapses to U = phi.et (8 accumulating
  matmuls, den rides along as extra ones-columns at partition 32), then the
  out conv contracts (tap, feature) = 12 terms via a padded [68,*] lhsT
  (partition blocks at 0/32/64 to satisfy the 32-partition alignment rule).
- branch b (q=clinical, k/v=image): scores = phi.h with h = Wq.k computed
  directly as a 4-row conv of image (shares the PE pass with g). v/ctx/out
  conv stay dense (baseline structure).
- BatchNorm batch stats per branch via AllReduce; branch b computed FIRST so
  its collective+finalize hide under branch a's compute; only branch a's
  collective is tail-exposed.
- residual comes from the bf16 imm tiles (no separate f32 image DMA).
"""
import os
import sys

sys.path.insert(0, '/opt/trn_rl_repo')

import ml_dtypes
import numpy as np

from concourse import bacc, mybir, tile
from concourse.bass_utils import run_bass_kernel_spmd

S = 2
NCORES = 8
C = 256
CT = 2
L = 1024
LS = 2
EPS = 1e-5
SCALE = 1.0 / 16.0
NSTAT = 16 * L

F32 = mybir.dt.float32
BF16 = mybir.dt.bfloat16
NPBF = ml_dtypes.bfloat16
AF = mybir.ActivationFunctionType
OP = mybir.AluOpType
AXX = mybir.AxisListType.X

_NC_CACHE = []


def _build_nc():
    nc = bacc.Bacc(num_devices=NCORES)

    imm_p = nc.declare_dram_parameter("imm", [S, CT, 128, L + 2], BF16, isOutput=False)
    cli_p = nc.declare_dram_parameter("cli", [S, 4, L], BF16, isOutput=False)
    clit_p = nc.declare_dram_parameter("clit", [S, 128, 288], BF16, isOutput=False)
    wgh_p = nc.declare_dram_parameter("wgh", [128, 216], BF16, isOutput=False)
    ghb_p = nc.declare_dram_parameter("ghb", [36, 1], F32, isOutput=False)
    owa_p = nc.declare_dram_parameter("owa", [68, 256], BF16, isOutput=False)
    wvb_p = nc.declare_dram_parameter("wvb", [CT, 3, 128, C], BF16, isOutput=False)
    vbb_p = nc.declare_dram_parameter("vbb", [128, C], F32, isOutput=False)
    wob_p = nc.declare_dram_parameter("wob", [CT, 128, 768], BF16, isOutput=False)
    bias_p = {}
    for name in ("oba", "obb", "ga_a", "be_a", "ga_b", "be_b"):
        bias_p[name] = nc.declare_dram_parameter(name, [128, CT], F32, isOutput=False)
    out_p = nc.declare_dram_parameter("out", [S, 2, CT, 128, L], F32, isOutput=True)

    from contextlib import ExitStack
    with tile.TileContext(nc) as tc, ExitStack() as es:
        ec = es.enter_context
        wgt = ec(tc.tile_pool(name="wgt", bufs=1))
        io = ec(tc.tile_pool(name="io", bufs=1))
        gs = ec(tc.tile_pool(name="gs", bufs=1))
        vtp = ec(tc.tile_pool(name="vtp", bufs=1))
        ex = ec(tc.tile_pool(name="ex", bufs=2))
        cx = ec(tc.tile_pool(name="cx", bufs=2))
        v3p = ec(tc.tile_pool(name="v3p", bufs=1))
        op_pool = ec(tc.tile_pool(name="op", bufs=1))
        sm = ec(tc.tile_pool(name="sm", bufs=2))
        sqp = ec(tc.tile_pool(name="sqp", bufs=2))
        st = ec(tc.tile_pool(name="st", bufs=1))
        bn = ec(tc.tile_pool(name="bn", bufs=3))
        dram = ec(tc.tile_pool(name="dram", bufs=1, space="DRAM"))
        ps_conv = ec(tc.tile_pool(name="psc", bufs=2, space="PSUM"))
        ps_sc = ec(tc.tile_pool(name="pss", bufs=3, space="PSUM"))
        ps_cu = ec(tc.tile_pool(name="psx", bufs=1, space="PSUM"))
        ps_den = ec(tc.tile_pool(name="psd", bufs=1, space="PSUM"))
        if True:
            # ---- DMAs, priority order ----
            wgh_sb = wgt.tile([128, 216], BF16, tag="wgh")
            nc.sync.dma_start(out=wgh_sb, in_=wgh_p[:, :])
            ghb_sb = wgt.tile([36, 1], F32, tag="ghb")
            nc.sync.dma_start(out=ghb_sb, in_=ghb_p[:, :])
            cli_sb = []
            for s in range(S):
                t_ = io.tile([4, L], BF16, tag=f"cli_{s}")
                nc.sync.dma_start(out=t_, in_=cli_p[s])
                cli_sb.append(t_)
            imm = []
            for s in range(S):
                imm.append([])
                for kt in range(CT):
                    t_ = io.tile([128, L + 2], BF16, tag=f"imm_{s}_{kt}")
                    nc.sync.dma_start(out=t_, in_=imm_p[s, kt])
                    imm[s].append(t_)
            wvb_sb = []
            for kt in range(CT):
                row = []
                for t in range(3):
                    t_ = wgt.tile([128, C], BF16, tag=f"wvb_{kt}_{t}")
                    nc.sync.dma_start(out=t_, in_=wvb_p[kt, t])
                    row.append(t_)
                wvb_sb.append(row)
            vbb_sb = wgt.tile([128, C], F32, tag="vbb")
            nc.sync.dma_start(out=vbb_sb, in_=vbb_p[:, :])
            clit_sb = []
            for s in range(S):
                t_ = io.tile([128, 288], BF16, tag=f"clit_{s}")
                nc.sync.dma_start(out=t_, in_=clit_p[s])
                clit_sb.append(t_)
            wob_sb = []
            for kt in range(CT):
                t_ = wgt.tile([128, 768], BF16, tag=f"wob_{kt}")
                nc.sync.dma_start(out=t_, in_=wob_p[kt])
                wob_sb.append(t_)
            owa_sb = wgt.tile([68, 256], BF16, tag="owa")
            nc.sync.dma_start(out=owa_sb, in_=owa_p[:, :])
            bias = {}
            for name in ("obb", "oba", "ga_b", "be_b", "ga_a", "be_a"):
                t_ = wgt.tile([128, CT], F32, tag=name)
                nc.sync.dma_start(out=t_, in_=bias_p[name][:, :])
                bias[name] = t_

            # ---- constants ----
            ones_full = wgt.tile([128, 128], BF16, tag="ones_full")
            nc.vector.memset(ones_full, 1.0)
            eps_sb = wgt.tile([128, 1], F32, tag="eps_sb")
            nc.vector.memset(eps_sb, EPS)
            zero_col = wgt.tile([128, 1], BF16, tag="zero_col")
            nc.vector.memset(zero_col, 0.0)

            # ---- g/h fused conv: one PE pass makes both 4-row convs ----
            g_sb = []
            h_sb = []
            for s in range(S):
                g_ = gs.tile([4, L], BF16, tag=f"g_{s}", name=f"g_{s}")
                h_ = gs.tile([4, L], BF16, tag=f"h_{s}", name=f"h_{s}")
                g_sb.append(g_)
                h_sb.append(h_)
            for s in range(S):
                for ls in range(LS):
                    p = ps_conv.tile([128, 512], F32, tag="conv", name="convp")
                    n = 0
                    for kt in range(CT):
                        for t in range(3):
                            nc.tensor.matmul(
                                p[0:36],
                                lhsT=wgh_sb[:, (kt * 3 + t) * 36:(kt * 3 + t + 1) * 36],
                                rhs=imm[s][kt][:, ls * 512 + t: ls * 512 + t + 512],
                                start=(n == 0), stop=(n == 5))
                            n += 1
                    nc.scalar.activation(
                        out=g_sb[s][:, ls * 512:(ls + 1) * 512], in_=p[0:4],
                        func=AF.Identity, bias=ghb_sb[0:4, 0:1], scale=1.0)
                    nc.scalar.activation(
                        out=h_sb[s][:, ls * 512:(ls + 1) * 512], in_=p[32:36],
                        func=AF.Identity, bias=ghb_sb[32:36, 0:1], scale=1.0)

            # ---- branch b v convs ----
            vt = {}
            for s in range(S):
                for mt in range(8):
                    p = ps_conv.tile([128, C], F32, tag="conv", name="convp")
                    n = 0
                    for kt in range(CT):
                        for t in range(3):
                            nc.tensor.matmul(
                                p,
                                lhsT=imm[s][kt][:, mt * 128 + t: mt * 128 + t + 128],
                                rhs=wvb_sb[kt][t],
                                start=(n == 0), stop=(n == 5))
                            n += 1
                    v_ = vtp.tile([128, C], BF16, tag=f"vt{s}_{mt}", name=f"vt{s}_{mt}")
                    nc.vector.tensor_add(out=v_, in0=p, in1=vbb_sb)
                    vt[(s, mt)] = v_

            o_tiles = {}
            slots = {}
            for br in range(2):
                slots[br] = st.tile([128, 4 * S * LS], F32, tag=f"slots{br}", name=f"slots{br}")

            # ---- branch b attention (dense v, rank-4 scores) ----
            def b_attention(s):
                ctx = [cx.tile([128, L + 2], BF16, tag=f"ctx{ct}", name=f"ctx{ct}") for ct in range(CT)]
                for ct in range(CT):
                    nc.vector.tensor_copy(out=ctx[ct][:, 0:1], in_=zero_col)
                    nc.vector.tensor_copy(out=ctx[ct][:, L + 1:L + 2], in_=zero_col)
                for ls in range(LS):
                    ets = {}

                    def _sc_exp(mt):
                        sc = ps_sc.tile([128, 512], F32, tag="sc", name="sc")
                        nc.tensor.matmul(
                            sc, lhsT=h_sb[s][:, mt * 128:(mt + 1) * 128],
                            rhs=cli_sb[s][:, ls * 512:(ls + 1) * 512],
                            start=True, stop=True)
                        et = ex.tile([128, 512], BF16, tag=f"et{mt}", name="et")
                        nc.scalar.activation(out=et, in_=sc, func=AF.Exp, scale=SCALE)
                        ets[mt] = et

                    ctx_ps = [ps_cu.tile([128, 512], F32, tag=f"ctxp{ct}", name=f"ctxp{ct}") for ct in range(CT)]
                    den_ps = ps_den.tile([128, 512], F32, tag="den", name="den_ps")
                    _sc_exp(0)
                    _sc_exp(1)
                    _sc_exp(2)
                    for mt in range(8):
                        if mt + 3 < 8:
                            _sc_exp(mt + 3)
                        et = ets.pop(mt)
                        for ct in range(CT):
                            nc.tensor.matmul(
                                ctx_ps[ct], lhsT=vt[(s, mt)][:, ct * 128:(ct + 1) * 128],
                                rhs=et, start=(mt == 0), stop=(mt == 7))
                        nc.tensor.matmul(
                            den_ps, lhsT=ones_full, rhs=et,
                            start=(mt == 0), stop=(mt == 7))
                    recip = sm.tile([128, 512], F32, tag="recip", name="recip")
                    nc.vector.reciprocal_approx_fast(out=recip, in_=den_ps)
                    for ct in range(CT):
                        nc.vector.tensor_mul(
                            out=ctx[ct][:, 1 + ls * 512: 1 + (ls + 1) * 512],
                            in0=ctx_ps[ct], in1=recip)
                return ctx

            def b_out_conv(s, ctx):
                for ct in range(CT):
                    o_sb = op_pool.tile([128, L], F32, tag=f"o_1_{s}_{ct}", name=f"o_1_{s}_{ct}")
                    o_tiles[(1, s, ct)] = o_sb
                    for ls in range(LS):
                        p = ps_conv.tile([128, 512], F32, tag="conv", name="convp")
                        n = 0
                        for kt in range(CT):
                            for t in range(3):
                                nc.tensor.matmul(
                                    p,
                                    lhsT=wob_sb[kt][:, (t * 2 + ct) * 128:(t * 2 + ct + 1) * 128],
                                    rhs=ctx[kt][:, ls * 512 + t: ls * 512 + t + 512],
                                    start=(n == 0), stop=(n == 5))
                                n += 1
                        osl = o_sb[:, ls * 512:(ls + 1) * 512]
                        i = ct * S * LS + s * LS + ls
                        nc.scalar.activation(
                            out=osl, in_=p, func=AF.Identity,
                            bias=bias["obb"][:, ct:ct + 1], scale=1.0,
                            accum_out=slots[1][:, i:i + 1])
                        sq = sqp.tile([128, 512], F32, tag="sq", name="sq")
                        nc.vector.tensor_mul(out=sq, in0=osl, in1=osl)
                        j = (2 + ct) * S * LS + s * LS + ls
                        nc.vector.reduce_sum(out=slots[1][:, j:j + 1], in_=sq, axis=AXX)

            # ---- branch a attention: U = phi.et with den columns ----
            def a_attention(s):
                v3 = v3p.tile([68, L + 2], BF16, tag=f"v3_{s}", name=f"v3_{s}")
                nc.vector.memset(v3, 0.0)
                for ls in range(LS):
                    ets = {}

                    def _sc_exp(mt):
                        sc = ps_sc.tile([128, 512], F32, tag="sc", name="sc")
                        nc.tensor.matmul(
                            sc, lhsT=cli_sb[s][:, mt * 128:(mt + 1) * 128],
                            rhs=g_sb[s][:, ls * 512:(ls + 1) * 512],
                            start=True, stop=True)
                        et = ex.tile([128, 512], BF16, tag=f"et{mt}", name="et")
                        nc.scalar.activation(out=et, in_=sc, func=AF.Exp, scale=SCALE)
                        ets[mt] = et

                    u_ps = ps_cu.tile([128, 512], F32, tag=f"ctxp{ls}", name="u_ps")
                    _sc_exp(0)
                    _sc_exp(1)
                    _sc_exp(2)
                    for mt in range(8):
                        if mt + 3 < 8:
                            _sc_exp(mt + 3)
                        et = ets.pop(mt)
                        nc.tensor.matmul(
                            u_ps[0:36], lhsT=clit_sb[s][:, mt * 36:(mt + 1) * 36],
                            rhs=et, start=(mt == 0), stop=(mt == 7))
                    rec4 = sm.tile([4, 512], F32, tag="rec4", name="rec4")
                    # NOTE: reciprocal_approx_fast drops the partition offset
                    # of its input AP; plain reciprocal handles base-32 reads.
                    nc.vector.reciprocal(out=rec4, in_=u_ps[32:36])
                    # V3 feature blocks at partitions 0/32/64; row 3 of each
                    # block is U[3]*rec = den/den = 1 (the folded v-bias lane)
                    for t in range(3):
                        c0 = ls * 512 + 2 - t
                        nc.vector.tensor_mul(
                            out=v3[32 * t:32 * t + 4, c0:c0 + 512],
                            in0=u_ps[0:4], in1=rec4)
                return v3

            def a_out_conv(s, v3):
                for ct in range(CT):
                    o_sb = op_pool.tile([128, L], F32, tag=f"o_0_{s}_{ct}", name=f"o_0_{s}_{ct}")
                    o_tiles[(0, s, ct)] = o_sb
                    for ls in range(LS):
                        p = ps_conv.tile([128, 512], F32, tag="conv", name="convp")
                        nc.tensor.matmul(
                            p, lhsT=owa_sb[:, ct * 128:(ct + 1) * 128],
                            rhs=v3[:, 1 + ls * 512: 1 + ls * 512 + 512],
                            start=True, stop=True)
                        osl = o_sb[:, ls * 512:(ls + 1) * 512]
                        i = ct * S * LS + s * LS + ls
                        nc.scalar.activation(
                            out=osl, in_=p, func=AF.Identity,
                            bias=bias["oba"][:, ct:ct + 1], scale=1.0,
                            accum_out=slots[0][:, i:i + 1])
                        sq = sqp.tile([128, 512], F32, tag="sq", name="sq")
                        nc.vector.tensor_mul(out=sq, in0=osl, in1=osl)
                        j = (2 + ct) * S * LS + s * LS + ls
                        nc.vector.reduce_sum(out=slots[0][:, j:j + 1], in_=sq, axis=AXX)

            def do_stats(br):
                statp = st.tile([128, 4], F32, tag=f"statp{br}", name=f"statp{br}")
                nc.vector.reduce_sum(
                    out=statp,
                    in_=slots[br].rearrange("p (g i) -> p g i", i=S * LS), axis=AXX)
                cc_in = dram.tile([128, 4], F32, tag=f"ccin{br}", name=f"ccin{br}")
                cc_out = dram.tile([128, 4], F32, tag=f"ccout{br}", name=f"ccout{br}")
                nc.sync.dma_start(out=cc_in, in_=statp)
                if os.environ.get("KERNEL_NO_CC"):
                    nc.sync.dma_start(out=cc_out, in_=cc_in)
                else:
                    nc.gpsimd.collective_compute(
                        "AllReduce", OP.add,
                        replica_groups=[list(range(NCORES))],
                        ins=[cc_in.opt()], outs=[cc_out.opt()])
                return cc_out

            # ---- schedule: branch b fully first, then branch a ----
            ctx0 = b_attention(0)
            b_out_conv(0, ctx0)
            ctx1 = b_attention(1)
            b_out_conv(1, ctx1)
            cc_b = do_stats(1)
            sg_b = st.tile([128, 4], F32, tag="sg_b", name="sg_b")
            nc.sync.dma_start(out=sg_b, in_=cc_b)

            v3_0 = a_attention(0)
            a_out_conv(0, v3_0)
            v3_1 = a_attention(1)
            a_out_conv(1, v3_1)
            cc_a = do_stats(0)

            # ---- BN coeffs + finalize ----
            def bn_coeffs(br, sg, gname, bname):
                mean = st.tile([128, CT], F32, tag=f"mean{br}", name=f"mean{br}")
                nc.vector.tensor_scalar_mul(mean, sg[:, 0:2], 1.0 / NSTAT)
                esq = st.tile([128, CT], F32, tag=f"esq{br}", name=f"esq{br}")
                nc.vector.tensor_scalar_mul(esq, sg[:, 2:4], 1.0 / NSTAT)
                m2 = st.tile([128, CT], F32, tag=f"m2{br}", name=f"m2{br}")
                nc.vector.tensor_mul(out=m2, in0=mean, in1=mean)
                var = st.tile([128, CT], F32, tag=f"var{br}", name=f"var{br}")
                nc.vector.tensor_sub(out=var, in0=esq, in1=m2)
                sd = st.tile([128, CT], F32, tag=f"sd{br}", name=f"sd{br}")
                nc.scalar.activation(out=sd, in_=var, func=AF.Sqrt, bias=eps_sb[:, 0:1], scale=1.0)
                rstd = st.tile([128, CT], F32, tag=f"rstd{br}", name=f"rstd{br}")
                nc.vector.reciprocal(out=rstd, in_=sd)
                A_ = st.tile([128, CT], F32, tag=f"A{br}", name=f"A{br}")
                nc.vector.tensor_mul(out=A_, in0=rstd, in1=bias[gname])
                mA = st.tile([128, CT], F32, tag=f"mA{br}", name=f"mA{br}")
                nc.vector.tensor_mul(out=mA, in0=mean, in1=A_)
                Bc = st.tile([128, CT], F32, tag=f"Bc{br}", name=f"Bc{br}")
                nc.vector.tensor_sub(out=Bc, in0=bias[bname], in1=mA)
                return A_, Bc

            def finalize(br, A_, Bc, on_scalar):
                for s in range(S):
                    for ct in range(CT):
                        tmp = bn.tile([128, L], F32, tag="bnt", name="bnt")
                        if on_scalar:
                            nc.scalar.activation(
                                out=tmp, in_=o_tiles[(br, s, ct)], func=AF.Identity,
                                scale=A_[:, ct:ct + 1], bias=Bc[:, ct:ct + 1])
                        else:
                            nc.vector.tensor_scalar(
                                out=tmp, in0=o_tiles[(br, s, ct)],
                                scalar1=A_[:, ct:ct + 1], scalar2=Bc[:, ct:ct + 1],
                                op0=OP.mult, op1=OP.add)
                        res_t = bn.tile([128, L], F32, tag="bnr", name="bnr")
                        nc.vector.tensor_add(
                            out=res_t, in0=tmp, in1=imm[s][ct][:, 1:L + 1])
                        nc.sync.dma_start(out=out_p[s, br, ct], in_=res_t)

            # finalize b on DVE (overlaps branch-a tail + AllReduce-a)
            A_b, B_b = bn_coeffs(1, sg_b, "ga_b", "be_b")
            finalize(1, A_b, B_b, on_scalar=False)
            sg_a = st.tile([128, 4], F32, tag="sg_a", name="sg_a")
            nc.sync.dma_start(out=sg_a, in_=cc_a)
            A_a, B_a = bn_coeffs(0, sg_a, "ga_a", "be_a")
            finalize(0, A_a, B_a, on_scalar=True)

    nc.compile()
    return nc


def _get_nc():
    if not _NC_CACHE:
        _NC_CACHE.append(_build_nc())
    return _NC_CACHE[0]


def _prep_shared(inp):
    f32 = np.float32

    def smat(w, b):
        # [4, C]: rows 0..2 = taps of the 1-in-channel conv, row 3 = bias
        return np.concatenate(
            [np.asarray(w, f32)[:, 0, :].T, np.asarray(b, f32)[None, :]], axis=0)

    Wk_a = smat(inp["a_kw"], inp["a_kb"])
    Wv_a = smat(inp["a_vw"], inp["a_vb"])
    Wq_b = smat(inp["b_qw"], inp["b_qb"])

    m = {}
    # fused 4-row convs: g = Wk_a.(conv(img, a_qw)+a_qb), h = Wq_b.(conv+kb)
    qw2 = np.einsum('ic,cjt->ijt', Wk_a, np.asarray(inp["a_qw"], f32))
    kw2 = np.einsum('ic,cjt->ijt', Wq_b, np.asarray(inp["b_kw"], f32))
    wgh = np.zeros((128, 216), f32)
    for kt in range(CT):
        for t in range(3):
            b0 = (kt * 3 + t) * 36
            wgh[:, b0:b0 + 4] = qw2[:, kt * 128:(kt + 1) * 128, t].T
            wgh[:, b0 + 32:b0 + 36] = kw2[:, kt * 128:(kt + 1) * 128, t].T
    m["wgh"] = wgh.astype(NPBF)
    ghb = np.zeros((36, 1), f32)
    ghb[0:4, 0] = Wk_a @ np.asarray(inp["a_qb"], f32)
    ghb[32:36, 0] = Wq_b @ np.asarray(inp["b_kb"], f32)
    m["ghb"] = ghb

    # branch a out conv folded through Wv_a: rows 32t+i = sum_ci ow[:,ci,t]*Wv[i,ci]
    ow2 = np.einsum('oct,ic->tio', np.asarray(inp["a_ow"], f32), Wv_a)
    owa = np.zeros((68, 256), f32)
    for t in range(3):
        owa[32 * t:32 * t + 4] = ow2[t]
    m["owa"] = owa.astype(NPBF)

    m["wvb"] = np.ascontiguousarray(
        np.asarray(inp["b_vw"], f32).reshape(C, 2, 128, 3).transpose(1, 3, 2, 0)).astype(NPBF)
    m["vbb"] = np.ascontiguousarray(
        np.repeat(np.asarray(inp["b_vb"], f32)[None, :], 128, axis=0))
    m["wob"] = np.ascontiguousarray(
        np.asarray(inp["b_ow"], f32).reshape(2, 128, 2, 128, 3).transpose(2, 3, 4, 0, 1).reshape(2, 128, 768)).astype(NPBF)
    for dst, src in (("oba", "a_ob"), ("obb", "b_ob"),
                     ("ga_a", "a_g"), ("be_a", "a_beta"),
                     ("ga_b", "b_g"), ("be_b", "b_beta")):
        m[dst] = np.ascontiguousarray(np.asarray(inp[src], f32).reshape(2, 128).T)
    return m


def _core_maps(image, clinical, shared, ncores=NCORES):
    in_maps = []
    for core in range(ncores):
        m = dict(shared)
        sl = slice(core * S, (core + 1) * S)
        a = image[sl].reshape(S, CT, 128, L)
        pad = np.zeros((S, CT, 128, L + 2), np.float32)
        pad[..., 1:L + 1] = a
        m["imm"] = pad.astype(NPBF)
        c = clinical[sl][:, 0, :]
        im2 = np.zeros((S, 4, L), np.float32)
        im2[:, 0, 1:] = c[:, :L - 1]
        im2[:, 1, :] = c
        im2[:, 2, :L - 1] = c[:, 1:]
        im2[:, 3, :] = 1.0
        m["cli"] = im2.astype(NPBF)
        clit = np.zeros((S, 128, 288), np.float32)
        for s in range(S):
            for mt in range(8):
                blk = clit[s, :, mt * 36:mt * 36 + 36]
                blk[:, 0:4] = im2[s, :, mt * 128:(mt + 1) * 128].T
                blk[:, 32:36] = 1.0
        m["clit"] = clit.astype(NPBF)
        in_maps.append(m)
    return in_maps


def kernel(**inputs):
    inp = {k: np.asarray(v) for k, v in inputs.items()}
    nc = _get_nc()
    shared = _prep_shared(inp)
    image = inp["image"].astype(np.float32)
    clinical = inp["clinical"].astype(np.float32)
    in_maps = _core_maps(image, clinical, shared)
    res = run_bass_kernel_spmd(nc, in_maps, core_ids=list(range(NCORES)))
    outs = np.concatenate([res.results[i]["out"] for i in range(NCORES)], axis=0)
    return np.ascontiguousarray(outs.reshape(16, 512, L))


# revision 8
# speedup vs baseline: 1.5338x; 1.2277x over previous
"""CrossSymmetricModal trn2 kernel v5: rank-4 factorization + branch-a-first
schedule.

Rank-4 structure (clinical is [B,1,L]): phi = (cli[m-1], cli[m], cli[m+1], 1).
- branch a: scores = g.phi (g = 4-row conv of image, weights folded on host),
  ctx collapses to U = phi.et with den riding as ones-columns at partition 32,
  out conv contracts (tap, feature) via a padded [68,*] lhsT (blocks at
  partitions 0/32/64 to satisfy 32-partition alignment).
- branch b: scores = phi.h (h shares the fused 4-row conv with g); v/ctx/out
  conv stay dense.

Schedule: branch a's attention is exp(scalar)-gated, so branch b's v-convs are
interleaved into it as PE filler; AllReduce-a then hides under branch b's
dense compute, and only AllReduce-b is tail-exposed. finalize-a runs on
DVE+gpsimd mid-branch-b; finalize-b (scalar+DVE) is the tail.
"""
import os
import sys

sys.path.insert(0, '/opt/trn_rl_repo')

import ml_dtypes
import numpy as np

from concourse import bacc, mybir, tile
from concourse.bass_utils import run_bass_kernel_spmd

S = 2
NCORES = 8
C = 256
CT = 2
L = 1024
LS = 2
EPS = 1e-5
SCALE = 1.0 / 16.0
NSTAT = 16 * L

F32 = mybir.dt.float32
BF16 = mybir.dt.bfloat16
NPBF = ml_dtypes.bfloat16
AF = mybir.ActivationFunctionType
OP = mybir.AluOpType
AXX = mybir.AxisListType.X

_NC_CACHE = []


def _build_nc():
    nc = bacc.Bacc(num_devices=NCORES)

    imm_p = nc.declare_dram_parameter("imm", [S, CT, 128, L + 2], BF16, isOutput=False)
    cli_p = nc.declare_dram_parameter("cli", [S, 4, L], BF16, isOutput=False)
    clit_p = nc.declare_dram_parameter("clit", [S, 128, 288], BF16, isOutput=False)
    wgh_p = nc.declare_dram_parameter("wgh", [128, 216], BF16, isOutput=False)
    ghb_p = nc.declare_dram_parameter("ghb", [36, 1], F32, isOutput=False)
    owa_p = nc.declare_dram_parameter("owa", [68, 256], BF16, isOutput=False)
    wvb_p = nc.declare_dram_parameter("wvb", [CT, 3, 128, C], BF16, isOutput=False)
    vbb_p = nc.declare_dram_parameter("vbb", [128, C], F32, isOutput=False)
    wob_p = nc.declare_dram_parameter("wob", [CT, 128, 768], BF16, isOutput=False)
    bias_p = {}
    for name in ("oba", "obb", "ga_a", "be_a", "ga_b", "be_b"):
        bias_p[name] = nc.declare_dram_parameter(name, [128, CT], F32, isOutput=False)
    out_p = nc.declare_dram_parameter("out", [S, 2, CT, 128, L], F32, isOutput=True)

    from contextlib import ExitStack
    with tile.TileContext(nc) as tc, ExitStack() as es:
        ec = es.enter_context
        wgt = ec(tc.tile_pool(name="wgt", bufs=1))
        io = ec(tc.tile_pool(name="io", bufs=1))
        gs = ec(tc.tile_pool(name="gs", bufs=1))
        vtp = ec(tc.tile_pool(name="vtp", bufs=1))
        ex = ec(tc.tile_pool(name="ex", bufs=2))
        cx = ec(tc.tile_pool(name="cx", bufs=2))
        v3p = ec(tc.tile_pool(name="v3p", bufs=1))
        op_pool = ec(tc.tile_pool(name="op", bufs=1))
        sm = ec(tc.tile_pool(name="sm", bufs=2))
        sqp = ec(tc.tile_pool(name="sqp", bufs=2))
        st = ec(tc.tile_pool(name="st", bufs=1))
        bn = ec(tc.tile_pool(name="bn", bufs=3))
        dram = ec(tc.tile_pool(name="dram", bufs=1, space="DRAM"))
        ps_conv = ec(tc.tile_pool(name="psc", bufs=2, space="PSUM"))
        ps_sc = ec(tc.tile_pool(name="pss", bufs=3, space="PSUM"))
        ps_cu = ec(tc.tile_pool(name="psx", bufs=1, space="PSUM"))
        ps_den = ec(tc.tile_pool(name="psd", bufs=1, space="PSUM"))
        if True:
            # ---- DMAs: first conv's inputs first ----
            imm = [[None] * CT for _ in range(S)]
            for kt in range(CT):
                t_ = io.tile([128, L + 2], BF16, tag=f"imm_0_{kt}", name=f"imm_0_{kt}")
                nc.sync.dma_start(out=t_, in_=imm_p[0, kt])
                imm[0][kt] = t_
            wgh_sb = wgt.tile([128, 216], BF16, tag="wgh")
            nc.sync.dma_start(out=wgh_sb, in_=wgh_p[:, :])
            ghb_sb = wgt.tile([36, 1], F32, tag="ghb")
            nc.sync.dma_start(out=ghb_sb, in_=ghb_p[:, :])
            wvb_sb = []
            for kt in range(CT):
                row = []
                for t in range(3):
                    t_ = wgt.tile([128, C], BF16, tag=f"wvb_{kt}_{t}")
                    nc.sync.dma_start(out=t_, in_=wvb_p[kt, t])
                    row.append(t_)
                wvb_sb.append(row)
            for kt in range(CT):
                t_ = io.tile([128, L + 2], BF16, tag=f"imm_1_{kt}", name=f"imm_1_{kt}")
                nc.sync.dma_start(out=t_, in_=imm_p[1, kt])
                imm[1][kt] = t_
            vbb_sb = wgt.tile([128, C], F32, tag="vbb")
            nc.sync.dma_start(out=vbb_sb, in_=vbb_p[:, :])
            cli_sb = []
            clit_sb = []
            for s in range(S):
                t_ = io.tile([4, L], BF16, tag=f"cli_{s}")
                nc.sync.dma_start(out=t_, in_=cli_p[s])
                cli_sb.append(t_)
                t2 = io.tile([128, 288], BF16, tag=f"clit_{s}")
                nc.sync.dma_start(out=t2, in_=clit_p[s])
                clit_sb.append(t2)
            owa_sb = wgt.tile([68, 256], BF16, tag="owa")
            nc.sync.dma_start(out=owa_sb, in_=owa_p[:, :])
            bias = {}
            for name in ("oba", "ga_a", "be_a"):
                t_ = wgt.tile([128, CT], F32, tag=name)
                nc.sync.dma_start(out=t_, in_=bias_p[name][:, :])
                bias[name] = t_
            wob_sb = []
            for kt in range(CT):
                t_ = wgt.tile([128, 768], BF16, tag=f"wob_{kt}")
                nc.sync.dma_start(out=t_, in_=wob_p[kt])
                wob_sb.append(t_)
            for name in ("obb", "ga_b", "be_b"):
                t_ = wgt.tile([128, CT], F32, tag=name)
                nc.sync.dma_start(out=t_, in_=bias_p[name][:, :])
                bias[name] = t_

            # ---- constants ----
            ones_full = wgt.tile([128, 128], BF16, tag="ones_full")
            nc.vector.memset(ones_full, 1.0)
            eps_sb = wgt.tile([128, 1], F32, tag="eps_sb")
            nc.vector.memset(eps_sb, EPS)
            zero_col = wgt.tile([128, 1], BF16, tag="zero_col")
            nc.vector.memset(zero_col, 0.0)

            # ---- g/h fused conv: one PE pass makes both 4-row convs ----
            g_sb = []
            h_sb = []
            for s in range(S):
                g_ = gs.tile([4, L], BF16, tag=f"g_{s}", name=f"g_{s}")
                h_ = gs.tile([4, L], BF16, tag=f"h_{s}", name=f"h_{s}")
                g_sb.append(g_)
                h_sb.append(h_)

            def gh_conv(s):
                for ls in range(LS):
                    p = ps_conv.tile([128, 512], F32, tag="conv", name="convp")
                    n = 0
                    for kt in range(CT):
                        for t in range(3):
                            nc.tensor.matmul(
                                p[0:36],
                                lhsT=wgh_sb[:, (kt * 3 + t) * 36:(kt * 3 + t + 1) * 36],
                                rhs=imm[s][kt][:, ls * 512 + t: ls * 512 + t + 512],
                                start=(n == 0), stop=(n == 5))
                            n += 1
                    nc.scalar.activation(
                        out=g_sb[s][:, ls * 512:(ls + 1) * 512], in_=p[0:4],
                        func=AF.Identity, bias=ghb_sb[0:4, 0:1], scale=1.0)
                    nc.scalar.activation(
                        out=h_sb[s][:, ls * 512:(ls + 1) * 512], in_=p[32:36],
                        func=AF.Identity, bias=ghb_sb[32:36, 0:1], scale=1.0)

            # ---- branch b v-conv units (PE filler inside branch a) ----
            vt = {}

            def bv_unit(s, mt):
                p = ps_conv.tile([128, C], F32, tag="conv", name="convp")
                n = 0
                for kt in range(CT):
                    for t in range(3):
                        nc.tensor.matmul(
                            p,
                            lhsT=imm[s][kt][:, mt * 128 + t: mt * 128 + t + 128],
                            rhs=wvb_sb[kt][t],
                            start=(n == 0), stop=(n == 5))
                        n += 1
                v_ = vtp.tile([128, C], BF16, tag=f"vt{s}_{mt}", name=f"vt{s}_{mt}")
                nc.vector.tensor_add(out=v_, in0=p, in1=vbb_sb)
                vt[(s, mt)] = v_

            fillers = [(s, mt) for s in range(S) for mt in range(8)]
            fill_i = [0]

            def fill(k):
                while k > 0 and fill_i[0] < len(fillers):
                    s, mt = fillers[fill_i[0]]
                    bv_unit(s, mt)
                    fill_i[0] += 1
                    k -= 1

            o_tiles = {}
            slots = {}
            for br in range(2):
                slots[br] = st.tile([128, 4 * S * LS], F32, tag=f"slots{br}", name=f"slots{br}")

            def sq_stat(br, osl, j):
                sq = sqp.tile([128, 512], F32, tag="sq", name="sq")
                nc.vector.scalar_tensor_tensor(
                    out=sq, in0=osl, scalar=1.0, in1=osl,
                    op0=OP.mult, op1=OP.mult,
                    accum_out=slots[br][:, j:j + 1])

            # ---- branch a attention: U = phi.et with den columns ----
            def a_attention(s):
                v3 = v3p.tile([68, L + 2], BF16, tag=f"v3_{s}", name=f"v3_{s}")
                nc.vector.memset(v3, 0.0)
                for ls in range(LS):
                    ets = {}

                    def _sc_exp(mt):
                        sc = ps_sc.tile([128, 512], F32, tag="sc", name="sc")
                        nc.tensor.matmul(
                            sc, lhsT=cli_sb[s][:, mt * 128:(mt + 1) * 128],
                            rhs=g_sb[s][:, ls * 512:(ls + 1) * 512],
                            start=True, stop=True)
                        et = ex.tile([128, 512], BF16, tag=f"et{mt}", name="et")
                        nc.scalar.activation(out=et, in_=sc, func=AF.Exp, scale=SCALE)
                        ets[mt] = et

                    u_ps = ps_cu.tile([128, 512], F32, tag=f"ctxp{ls}", name="u_ps")
                    _sc_exp(0)
                    _sc_exp(1)
                    _sc_exp(2)
                    for mt in range(8):
                        if mt + 3 < 8:
                            _sc_exp(mt + 3)
                        et = ets.pop(mt)
                        nc.tensor.matmul(
                            u_ps[0:36], lhsT=clit_sb[s][:, mt * 36:(mt + 1) * 36],
                            rhs=et, start=(mt == 0), stop=(mt == 7))
                        fill(1)
                    den4 = sm.tile([4, 512], F32, tag="den4", name="den4")
                    nc.vector.tensor_copy(out=den4, in_=u_ps[32:36])
                    rec4 = sm.tile([4, 512], F32, tag="rec4", name="rec4")
                    nc.vector.reciprocal_approx_fast(out=rec4, in_=den4)
                    # V3 feature blocks at partitions 0/32/64; row 3 of each
                    # block is U[3]*rec = den/den = 1 (the folded v-bias lane)
                    for t in range(3):
                        c0 = ls * 512 + 2 - t
                        nc.vector.tensor_mul(
                            out=v3[32 * t:32 * t + 4, c0:c0 + 512],
                            in0=u_ps[0:4], in1=rec4)
                return v3

            def a_out_conv(s, v3):
                for ct in range(CT):
                    o_sb = op_pool.tile([128, L], F32, tag=f"o_0_{s}_{ct}", name=f"o_0_{s}_{ct}")
                    o_tiles[(0, s, ct)] = o_sb
                    for ls in range(LS):
                        p = ps_conv.tile([128, 512], F32, tag="conv", name="convp")
                        nc.tensor.matmul(
                            p, lhsT=owa_sb[:, ct * 128:(ct + 1) * 128],
                            rhs=v3[:, 1 + ls * 512: 1 + ls * 512 + 512],
                            start=True, stop=True)
                        osl = o_sb[:, ls * 512:(ls + 1) * 512]
                        i = ct * S * LS + s * LS + ls
                        nc.scalar.activation(
                            out=osl, in_=p, func=AF.Identity,
                            bias=bias["oba"][:, ct:ct + 1], scale=1.0,
                            accum_out=slots[0][:, i:i + 1])
                        sq_stat(0, osl, (2 + ct) * S * LS + s * LS + ls)

            # ---- branch b attention (dense v, rank-4 scores) ----
            def b_attention(s):
                ctx = [cx.tile([128, L + 2], BF16, tag=f"ctx{ct}", name=f"ctx{ct}") for ct in range(CT)]
                for ct in range(CT):
                    nc.vector.tensor_copy(out=ctx[ct][:, 0:1], in_=zero_col)
                    nc.vector.tensor_copy(out=ctx[ct][:, L + 1:L + 2], in_=zero_col)
                for ls in range(LS):
                    ets = {}

                    def _sc_exp(mt):
                        sc = ps_sc.tile([128, 512], F32, tag="sc", name="sc")
                        nc.tensor.matmul(
                            sc, lhsT=h_sb[s][:, mt * 128:(mt + 1) * 128],
                            rhs=cli_sb[s][:, ls * 512:(ls + 1) * 512],
                            start=True, stop=True)
                        et = ex.tile([128, 512], BF16, tag=f"et{mt}", name="et")
                        nc.scalar.activation(out=et, in_=sc, func=AF.Exp, scale=SCALE)
                        ets[mt] = et

                    ctx_ps = [ps_cu.tile([128, 512], F32, tag=f"ctxp{ct}", name=f"ctxp{ct}") for ct in range(CT)]
                    den_ps = ps_den.tile([128, 512], F32, tag="den", name="den_ps")
                    _sc_exp(0)
                    _sc_exp(1)
                    _sc_exp(2)
                    for mt in range(8):
                        if mt + 3 < 8:
                            _sc_exp(mt + 3)
                        et = ets.pop(mt)
                        for ct in range(CT):
                            nc.tensor.matmul(
                                ctx_ps[ct], lhsT=vt[(s, mt)][:, ct * 128:(ct + 1) * 128],
                                rhs=et, start=(mt == 0), stop=(mt == 7))
                        nc.tensor.matmul(
                            den_ps, lhsT=ones_full, rhs=et,
                            start=(mt == 0), stop=(mt == 7))
                    recip = sm.tile([128, 512], F32, tag="recip", name="recip")
                    nc.vector.reciprocal_approx_fast(out=recip, in_=den_ps)
                    for ct in range(CT):
                        nc.vector.tensor_mul(
                            out=ctx[ct][:, 1 + ls * 512: 1 + (ls + 1) * 512],
                            in0=ctx_ps[ct], in1=recip)
                return ctx

            def b_out_conv(s, ctx):
                for ct in range(CT):
                    o_sb = op_pool.tile([128, L], F32, tag=f"o_1_{s}_{ct}", name=f"o_1_{s}_{ct}")
                    o_tiles[(1, s, ct)] = o_sb
                    for ls in range(LS):
                        p = ps_conv.tile([128, 512], F32, tag="conv", name="convp")
                        n = 0
                        for kt in range(CT):
                            for t in range(3):
                                nc.tensor.matmul(
                                    p,
                                    lhsT=wob_sb[kt][:, (t * 2 + ct) * 128:(t * 2 + ct + 1) * 128],
                                    rhs=ctx[kt][:, ls * 512 + t: ls * 512 + t + 512],
                                    start=(n == 0), stop=(n == 5))
                                n += 1
                        osl = o_sb[:, ls * 512:(ls + 1) * 512]
                        i = ct * S * LS + s * LS + ls
                        nc.scalar.activation(
                            out=osl, in_=p, func=AF.Identity,
                            bias=bias["obb"][:, ct:ct + 1], scale=1.0,
                            accum_out=slots[1][:, i:i + 1])
                        sq_stat(1, osl, (2 + ct) * S * LS + s * LS + ls)

            def do_stats(br):
                statp = st.tile([128, 4], F32, tag=f"statp{br}", name=f"statp{br}")
                nc.vector.reduce_sum(
                    out=statp,
                    in_=slots[br].rearrange("p (g i) -> p g i", i=S * LS), axis=AXX)
                cc_in = dram.tile([128, 4], F32, tag=f"ccin{br}", name=f"ccin{br}")
                cc_out = dram.tile([128, 4], F32, tag=f"ccout{br}", name=f"ccout{br}")
                nc.sync.dma_start(out=cc_in, in_=statp)
                if os.environ.get("KERNEL_NO_CC"):
                    nc.sync.dma_start(out=cc_out, in_=cc_in)
                else:
                    nc.gpsimd.collective_compute(
                        "AllReduce", OP.add,
                        replica_groups=[list(range(NCORES))],
                        ins=[cc_in.opt()], outs=[cc_out.opt()])
                return cc_out

            def bn_coeffs(br, sg, gname, bname):
                mean = st.tile([128, CT], F32, tag=f"mean{br}", name=f"mean{br}")
                nc.vector.tensor_scalar_mul(mean, sg[:, 0:2], 1.0 / NSTAT)
                esq = st.tile([128, CT], F32, tag=f"esq{br}", name=f"esq{br}")
                nc.vector.tensor_scalar_mul(esq, sg[:, 2:4], 1.0 / NSTAT)
                m2 = st.tile([128, CT], F32, tag=f"m2{br}", name=f"m2{br}")
                nc.vector.tensor_mul(out=m2, in0=mean, in1=mean)
                var = st.tile([128, CT], F32, tag=f"var{br}", name=f"var{br}")
                nc.vector.tensor_sub(out=var, in0=esq, in1=m2)
                sd = st.tile([128, CT], F32, tag=f"sd{br}", name=f"sd{br}")
                nc.scalar.activation(out=sd, in_=var, func=AF.Sqrt, bias=eps_sb[:, 0:1], scale=1.0)
                rstd = st.tile([128, CT], F32, tag=f"rstd{br}", name=f"rstd{br}")
                nc.vector.reciprocal(out=rstd, in_=sd)
                A_ = st.tile([128, CT], F32, tag=f"A{br}", name=f"A{br}")
                nc.vector.tensor_mul(out=A_, in0=rstd, in1=bias[gname])
                mA = st.tile([128, CT], F32, tag=f"mA{br}", name=f"mA{br}")
                nc.vector.tensor_mul(out=mA, in0=mean, in1=A_)
                Bc = st.tile([128, CT], F32, tag=f"Bc{br}", name=f"Bc{br}")
                nc.vector.tensor_sub(out=Bc, in0=bias[bname], in1=mA)
                return A_, Bc

            def finalize(br, A_, Bc):
                # affine on scalar ACT, residual add on DVE
                for s in range(S):
                    for ct in range(CT):
                        tmp = bn.tile([128, L], F32, tag="bnt", name="bnt")
                        nc.scalar.activation(
                            out=tmp, in_=o_tiles[(br, s, ct)], func=AF.Identity,
                            scale=A_[:, ct:ct + 1], bias=Bc[:, ct:ct + 1])
                        res_t = bn.tile([128, L], F32, tag="bnr", name="bnr")
                        nc.vector.tensor_add(
                            out=res_t, in0=tmp, in1=imm[s][ct][:, 1:L + 1])
                        nc.sync.dma_start(out=out_p[s, br, ct], in_=res_t)

            # ---- schedule ----
            gh_conv(0)
            gh_conv(1)
            v3_0 = a_attention(0)   # bv units for s0 fill the exp gaps
            v3_1 = a_attention(1)
            a_out_conv(0, v3_0)
            a_out_conv(1, v3_1)
            fill(99)                # any bv units not yet emitted
            cc_a = do_stats(0)
            sg_a = st.tile([128, 4], F32, tag="sg_a", name="sg_a")
            nc.sync.dma_start(out=sg_a, in_=cc_a)

            ctx0 = b_attention(0)
            b_out_conv(0, ctx0)
            ctx1 = b_attention(1)
            b_out_conv(1, ctx1)

            # stats-b reduce + DMA first so AR-b can trigger promptly
            statp_b = st.tile([128, 4], F32, tag="statp1", name="statp1")
            nc.vector.reduce_sum(
                out=statp_b,
                in_=slots[1].rearrange("p (g i) -> p g i", i=S * LS), axis=AXX)
            cc_in_b = dram.tile([128, 4], F32, tag="ccin1", name="ccin1")
            nc.sync.dma_start(out=cc_in_b, in_=statp_b)

            # finalize a (hidden under branch b tail / AR-b): DVE + gpsimd
            A_a, B_a = bn_coeffs(0, sg_a, "ga_a", "be_a")
            finalize(0, A_a, B_a)

            # AR-b (gpsimd queue: after finalize-a adds)
            cc_out_b = dram.tile([128, 4], F32, tag="ccout1", name="ccout1")
            if os.environ.get("KERNEL_NO_CC"):
                nc.sync.dma_start(out=cc_out_b, in_=cc_in_b)
            else:
                nc.gpsimd.collective_compute(
                    "AllReduce", OP.add,
                    replica_groups=[list(range(NCORES))],
                    ins=[cc_in_b.opt()], outs=[cc_out_b.opt()])
            sg_b = st.tile([128, 4], F32, tag="sg_b", name="sg_b")
            nc.sync.dma_start(out=sg_b, in_=cc_out_b)
            A_b, B_b = bn_coeffs(1, sg_b, "ga_b", "be_b")
            finalize(1, A_b, B_b)

    nc.compile()
    return nc


def _get_nc():
    if not _NC_CACHE:
        _NC_CACHE.append(_build_nc())
    return _NC_CACHE[0]


def _prep_shared(inp):
    f32 = np.float32

    def smat(w, b):
        # [4, C]: rows 0..2 = taps of the 1-in-channel conv, row 3 = bias
        return np.concatenate(
            [np.asarray(w, f32)[:, 0, :].T, np.asarray(b, f32)[None, :]], axis=0)

    Wk_a = smat(inp["a_kw"], inp["a_kb"])
    Wv_a = smat(inp["a_vw"], inp["a_vb"])
    Wq_b = smat(inp["b_qw"], inp["b_qb"])

    m = {}
    # fused 4-row convs: g = Wk_a.(conv(img, a_qw)+a_qb), h = Wq_b.(conv+kb)
    qw2 = np.einsum('ic,cjt->ijt', Wk_a, np.asarray(inp["a_qw"], f32))
    kw2 = np.einsum('ic,cjt->ijt', Wq_b, np.asarray(inp["b_kw"], f32))
    wgh = np.zeros((128, 216), f32)
    for kt in range(CT):
        for t in range(3):
            b0 = (kt * 3 + t) * 36
            wgh[:, b0:b0 + 4] = qw2[:, kt * 128:(kt + 1) * 128, t].T
            wgh[:, b0 + 32:b0 + 36] = kw2[:, kt * 128:(kt + 1) * 128, t].T
    m["wgh"] = wgh.astype(NPBF)
    ghb = np.zeros((36, 1), f32)
    ghb[0:4, 0] = Wk_a @ np.asarray(inp["a_qb"], f32)
    ghb[32:36, 0] = Wq_b @ np.asarray(inp["b_kb"], f32)
    m["ghb"] = ghb

    # branch a out conv folded through Wv_a: rows 32t+i = sum_ci ow[:,ci,t]*Wv[i,ci]
    ow2 = np.einsum('oct,ic->tio', np.asarray(inp["a_ow"], f32), Wv_a)
    owa = np.zeros((68, 256), f32)
    for t in range(3):
        owa[32 * t:32 * t + 4] = ow2[t]
    m["owa"] = owa.astype(NPBF)

    m["wvb"] = np.ascontiguousarray(
        np.asarray(inp["b_vw"], f32).reshape(C, 2, 128, 3).transpose(1, 3, 2, 0)).astype(NPBF)
    m["vbb"] = np.ascontiguousarray(
        np.repeat(np.asarray(inp["b_vb"], f32)[None, :], 128, axis=0))
    m["wob"] = np.ascontiguousarray(
        np.asarray(inp["b_ow"], f32).reshape(2, 128, 2, 128, 3).transpose(2, 3, 4, 0, 1).reshape(2, 128, 768)).astype(NPBF)
    for dst, src in (("oba", "a_ob"), ("obb", "b_ob"),
                     ("ga_a", "a_g"), ("be_a", "a_beta"),
                     ("ga_b", "b_g"), ("be_b", "b_beta")):
        m[dst] = np.ascontiguousarray(np.asarray(inp[src], f32).reshape(2, 128).T)
    return m


def _core_maps(image, clinical, shared, ncores=NCORES):
    in_maps = []
    for core in range(ncores):
        m = dict(shared)
        sl = slice(core * S, (core + 1) * S)
        a = image[sl].reshape(S, CT, 128, L)
        pad = np.zeros((S, CT, 128, L + 2), np.float32)
        pad[..., 1:L + 1] = a
        m["imm"] = pad.astype(NPBF)
        c = clinical[sl][:, 0, :]
        im2 = np.zeros((S, 4, L), np.float32)
        im2[:, 0, 1:] = c[:, :L - 1]
        im2[:, 1, :] = c
        im2[:, 2, :L - 1] = c[:, 1:]
        im2[:, 3, :] = 1.0
        m["cli"] = im2.astype(NPBF)
        clit = np.zeros((S, 128, 288), np.float32)
        for s in range(S):
            for mt in range(8):
                blk = clit[s, :, mt * 36:mt * 36 + 36]
                blk[:, 0:4] = im2[s, :, mt * 128:(mt + 1) * 128].T
                blk[:, 32:36] = 1.0
        m["clit"] = clit.astype(NPBF)
        in_maps.append(m)
    return in_maps


def kernel(**inputs):
    inp = {k: np.asarray(v) for k, v in inputs.items()}
    nc = _get_nc()
    shared = _prep_shared(inp)
    image = inp["image"].astype(np.float32)
    clinical = inp["clinical"].astype(np.float32)
    in_maps = _core_maps(image, clinical, shared)
    res = run_bass_kernel_spmd(nc, in_maps, core_ids=list(range(NCORES)))
    outs = np.concatenate([res.results[i]["out"] for i in range(NCORES)], axis=0)
    return np.ascontiguousarray(outs.reshape(16, 512, L))
